# revision 14
# baseline (speedup 1.0000x reference)
"""Trainium2 Bass kernel for the DEN (Mahalanobis distance) layer.

Computes out[b, e] = (x_b - c_e)^T Sigma_e^{-1} (x_b - c_e) for
x [8192, 128], Centroids [128, 1, 128], Sigmas [128, 128, 128].

Strategy (v7: fp16-highbyte fp8 DoubleRow, arrival-ordered cover)
-----------------------------------------------------------------
Wrapped-diagonal decomposition.  The 64 off-diagonal product packs are
elementwise products of rotated copies of x; a 15-slot difference cover
(rotations 0..7 and 16,24,...,64; diagonal 8 comes from the 16x24 pair)
is DMAed as bf16, pre-scaled by 1/4 on the host so products land at p/16.

DVE writes 56 packs as fp16 at the 2x-mode rate (~0.55 ns/elem) in 20
small batches ordered to match DMA chunk arrival; the idle GpSimd writes
the last 8 (diagonals 57..64).  The PE reads pack HIGH BYTES through a
stride-2 fp8e5 view (fp16 truncation == e5m2) and consumes PAIRS of packs
per fp8 DoubleRow matmul with e4m3 coefficients (x16, CCOMP compensates
the truncation bias).  Everything accumulates into one f32 PSUM group
(2 banks) together with the bf16 linear/x^2 packs; Act evicts bank 0 with
bias=tv while the bank-1 tail finishes, DVE evicts bank 1.

Sharding: data-parallel over batch B across the 8 cores (1024 rows each);
coefficients replicated.
"""

import os
import sys

sys.path.insert(0, "/opt/trn_rl_repo")

import numpy as np
import ml_dtypes

E, B, D = 128, 8192, 128
NCORES = 8
BLOC = B // NCORES          # 1024 batch rows per core
BT = 512                    # matmul free-dim tile (one PSUM bank)
NSLOT = 11                  # rotation slots 0..7, then 16,24,32
ROTVALS = tuple(range(8)) + (16, 24, 32)
NWARM = 5
NPAIR = 32                  # 64 off-diag packs -> 32 DoubleRow pairs
CCOMP = 1.06                # e5m2 truncation compensation (host-tuned)

# DVE product batches (in0 slot range [lo,hi) packed, in1 slot broadcast),
# ordered by DMA chunk arrival.  qSP carries slots 0,2,4,6,14; qAct carries
# 1,8,3,9,5,10,7,11,12,13 (one 256KB chunk each, interleaved with b-slots).
DVE_BATCHES = [
    (1, 2, 0),      # diag 1
    (0, 2, 8),      # 16,15
    (2, 3, 0),      # 2
    (2, 3, 8),      # 14
    (3, 4, 0),      # 3
    (4, 5, 0),      # 4
    (0, 3, 9),      # 24,23,22
    (8, 9, 9),      # 8   (16x24)
    (8, 10, 3),     # 13,21
    (0, 4, 10),     # 32..29
    (8, 11, 4),     # 12,20,28
    (5, 6, 0),      # 5
    (8, 11, 5),     # 11,19,27
    (6, 7, 0),      # 6
    (8, 11, 6),     # 10,18,26
    (7, 8, 0),      # 7
    (8, 11, 7),     # 9,17,25
]
# 32 packs shipped from host as round-nearest fp8e5 (pairs 16..31)
HOST_DIAGS = list(range(33, 65))
N_HOST = len(HOST_DIAGS)

PAIRS_AB = []
for _lo, _hi, _s1 in DVE_BATCHES:
    for _i in range(_lo, _hi):
        PAIRS_AB.append((ROTVALS[_i], ROTVALS[_s1]))
PAIRS_AB += [(0, j) for j in HOST_DIAGS]
assert len(PAIRS_AB) == 2 * NPAIR
_djs = sorted(min((b - a) % 128, (a - b) % 128) for a, b in PAIRS_AB)
assert _djs == list(range(1, 65)), _djs
N_DVE = sum(hi - lo for lo, hi, _ in DVE_BATCHES)       # 64

bf16 = ml_dtypes.bfloat16
f8e4m3 = ml_dtypes.float8_e4m3

_STATE: dict = {}


def _build_module():
    import concourse.bacc as bacc
    import concourse.tile as tile
    import concourse.mybir as mybir
    from contextlib import ExitStack

    nc = bacc.Bacc("TRN2", target_bir_lowering=False, debug=False)

    xr_d = nc.dram_tensor("xrot", [D, NSLOT * BLOC], mybir.dt.bfloat16,
                          kind="ExternalInput")
    # cwbt: [linear(E) | diag(E) | tv as 2 bf16 halves] per partition row
    cwbt_d = nc.dram_tensor("cwbt", [D, 2 * E + 2], mybir.dt.bfloat16,
                            kind="ExternalInput")
    cwf_d = nc.dram_tensor("cwf", [D, NPAIR * 2 * E], mybir.dt.float8e4,
                           kind="ExternalInput")
    hp_d = nc.dram_tensor("hp", [D, N_HOST * BLOC], mybir.dt.float8e5,
                          kind="ExternalInput")
    out_d = nc.dram_tensor("out", [E, BLOC], mybir.dt.bfloat16,
                           kind="ExternalOutput")

    f32 = mybir.dt.float32
    b16 = mybir.dt.bfloat16
    f16 = mybir.dt.float16
    f8e5 = mybir.dt.float8e5
    Ident = mybir.ActivationFunctionType.Identity
    DR = mybir.MatmulPerfMode.DoubleRow

    with tile.TileContext(nc) as tc, ExitStack() as ctx:
        const_pool = ctx.enter_context(tc.tile_pool(name="const", bufs=1))
        psum_pool = ctx.enter_context(tc.tile_pool(name="acc", bufs=2, space="PSUM"))

        # PE warmup on a GpSimd-memset tile: no DMA dependency, trips the
        # clock gate during the DMA prologue.
        WU = const_pool.tile([D, BT], b16, tag="warm")
        nc.gpsimd.memset(WU[:, :], 0)
        PSW = psum_pool.tile([E, BT], f32, tag="psw", name="psw")
        for _ in range(NWARM):
            nc.tensor.matmul(PSW[:, :], WU[:, 0:E], WU[:, :],
                             start=True, stop=True, skip_group_check=True)

        ROTS = const_pool.tile([D, NSLOT * BLOC], b16, tag="rots")
        R3 = ROTS[:, :].rearrange("p (s b) -> p s b", s=NSLOT)
        H = const_pool.tile([D, 32 * BLOC], f16, tag="H")
        H3 = H[:, :].rearrange("p (k b) -> p k b", k=32)
        X2 = const_pool.tile([D, BLOC], b16, tag="x2")
        CWBT = const_pool.tile([D, 2 * E + 2], b16, tag="cwbt")
        TV = CWBT[:, 2 * E:2 * E + 2].bitcast(f32)      # [128, 1] f32 bias
        CWF = const_pool.tile([D, NPAIR * 2 * E], mybir.dt.float8e4, tag="cwf")
        CWF4 = CWF[:, :].rearrange("p (t two e) -> p t two e", t=NPAIR, two=2)
        OT = const_pool.tile([E, BLOC], b16, tag="ot")
        HP = const_pool.tile([D, N_HOST * BLOC], f8e5, tag="hp")
        HP4 = HP[:, :].rearrange("p (t two n) -> p t two n", t=N_HOST // 2, two=2)

        # stride-2 fp8e5 view of H high bytes: [D, pack, col, (lo,hi)] -> hi
        He5 = H[:, :].bitcast(f8e5)
        HV = He5.rearrange("p (k b s) -> p k b s", k=32, s=2)[:, :, :, 1:2]

        def dma_rots(eng, lo, hi):
            eng.dma_start(ROTS[:, lo * BLOC:hi * BLOC],
                          xr_d.ap()[:, lo * BLOC:hi * BLOC])

        # DMA schedule: single-slot chunks, interleaved so DVE batches
        # unlock in arrival order.
        c0 = 8 * 2 * E          # cwf pairs 0..7
        c1 = 16 * 2 * E         # cwf pairs 8..15
        dma_rots(nc.sync, 0, 1)
        dma_rots(nc.scalar, 1, 2)
        dma_rots(nc.sync, 2, 3)
        dma_rots(nc.scalar, 8, 9)
        dma_rots(nc.sync, 4, 5)
        dma_rots(nc.scalar, 3, 4)
        dma_rots(nc.sync, 6, 7)
        dma_rots(nc.scalar, 9, 10)
        nc.sync.dma_start(CWF[:, 0:c0], cwf_d.ap()[:, 0:c0])
        nc.scalar.dma_start(CWF[:, c0:c1], cwf_d.ap()[:, c0:c1])
        nc.sync.dma_start(CWBT[:, :], cwbt_d.ap())
        dma_rots(nc.scalar, 10, 11)
        h8 = 8 * BLOC
        nc.sync.dma_start(HP[:, 0:h8], hp_d.ap()[:, 0:h8])
        dma_rots(nc.scalar, 5, 6)
        nc.sync.dma_start(CWF[:, c1:], cwf_d.ap()[:, c1:])
        nc.scalar.dma_start(HP[:, h8:2 * h8], hp_d.ap()[:, h8:2 * h8])
        nc.sync.dma_start(HP[:, 3 * h8:], hp_d.ap()[:, 3 * h8:])
        dma_rots(nc.scalar, 7, 8)
        nc.scalar.dma_start(HP[:, 2 * h8:3 * h8], hp_d.ap()[:, 2 * h8:3 * h8])

        # x^2 on the otherwise-idle GpSimd
        nc.gpsimd.tensor_mul(X2[:, :], ROTS[:, 0:BLOC], ROTS[:, 0:BLOC])

        PS = psum_pool.tile([E, BLOC], f32, tag="ps", name="ps")

        pos = 0         # next pack position not yet consumed by the PE
        hpos = [N_DVE // 2]          # next host pair to emit

        NDP = N_DVE // 2            # DVE-fed pairs

        def emit_host_pair(t, halves, stop):
            for h in halves:
                rhs = HP4[:, t - NDP, :, h * BT:(h + 1) * BT]
                nc.tensor.matmul(PS[:, h * BT:(h + 1) * BT],
                                 CWF4[:, t], rhs,
                                 start=False, stop=stop, perf_mode=DR)

        def emit_pair(t, halves, stop):
            for h in halves:
                rhs = HV[:, 2 * t:2 * t + 2,
                         h * BT:(h + 1) * BT, :].rearrange(
                             "p two b one -> p two (b one)")
                nc.tensor.matmul(PS[:, h * BT:(h + 1) * BT],
                                 CWF4[:, t], rhs,
                                 start=(t == 0),
                                 stop=stop, perf_mode=DR)

        def emit_pairs_until(limit):
            nonlocal pos
            while pos + 2 <= limit:
                t = pos // 2
                emit_pair(t, (0, 1), stop=False)
                pos += 2
                if pos >= 22 and hpos[0] < N_DVE // 2 + 8:
                    emit_host_pair(hpos[0], (0, 1), stop=False)
                    hpos[0] += 1
                if pos == 8:
                    # linear pack (rhs = x/4) mid-chain
                    for bt in range(2):
                        nc.tensor.matmul(PS[:, bt * BT:(bt + 1) * BT],
                                         CWBT[:, 0:E],
                                         ROTS[:, bt * BT:(bt + 1) * BT],
                                         start=False, stop=False)
                if pos == 16:
                    # x^2 pack mid-chain
                    for bt in range(2):
                        nc.tensor.matmul(PS[:, bt * BT:(bt + 1) * BT],
                                         CWBT[:, E:2 * E],
                                         X2[:, bt * BT:(bt + 1) * BT],
                                         start=False, stop=False)

        gpos = 0
        for lo, hi, s1 in DVE_BATCHES:
            w = hi - lo
            nc.vector.tensor_mul(
                H3[:, gpos:gpos + w, :],
                R3[:, lo:hi, :],
                R3[:, s1:s1 + 1, :].broadcast_to((D, w, BLOC)),
            )
            gpos += w
            emit_pairs_until(gpos)
        assert gpos == N_DVE
        # remaining host-fed pairs: all h0s first (bank 0 closes early,
        # eviction overlaps the h1 tail), then all h1s
        for t in range(hpos[0], NPAIR):
            emit_host_pair(t, (0,), stop=(t == NPAIR - 1))

        # bank 0 evicts on Act while bank-1 h1s run, bank 1 on DVE
        nc.scalar.activation(OT[:, 0:BT], PS[:, 0:BT], Ident, bias=TV[:, 0:1])
        nc.sync.dma_start(out_d.ap()[:, 0:BT], OT[:, 0:BT])
        for t in range(hpos[0], NPAIR):
            emit_host_pair(t, (1,), stop=(t == NPAIR - 1))
        nc.vector.tensor_scalar_add(OT[:, BT:BLOC], PS[:, BT:BLOC], TV[:, 0:1])
        nc.scalar.dma_start(out_d.ap()[:, BT:BLOC], OT[:, BT:BLOC])

    nc.compile()
    return nc


def _host_precompute(Centroids: np.ndarray, Sigmas: np.ndarray):
    Sinv = np.linalg.inv(Sigmas.astype(np.float64))
    A = 0.5 * (Sinv + np.swapaxes(Sinv, 1, 2))          # [E, D, D]
    c = Centroids[:, 0, :].astype(np.float64)           # [E, D]
    Ac = np.einsum("edk,ek->ed", A, c)

    idx = np.arange(D)
    cwb = np.zeros((D, 2 * E + 2), np.float32)
    cwb[:, 0:E] = (-2.0 * 4.0 * Ac.T)                   # rhs is x/4
    cwb[:, E:2 * E] = 16.0 * A[:, idx, idx].T           # rhs is (x/4)^2
    cwbt_host = np.ascontiguousarray(cwb).astype(bf16)
    tv = np.einsum("ed,ed->e", Ac, c).astype(np.float32)
    cwbt_host[:, 2 * E:2 * E + 2] = np.ascontiguousarray(
        tv[:, None]).view(bf16).reshape(D, 2)

    cwf = np.zeros((D, 64, E), np.float32)
    for k, (a, b) in enumerate(PAIRS_AB):
        dj = (b - a) % 128
        s = 1.0 if dj == 64 else 2.0
        comp = CCOMP if k < N_DVE else 1.0      # host packs are round-nearest
        cwf[:, k, :] = (s * 16.0 * comp) * A[:, (idx + a) % D,
                                             (idx + b) % D].T
    cwf_host = np.ascontiguousarray(cwf.reshape(D, NPAIR * 2 * E)).astype(f8e4m3)
    return cwbt_host, cwf_host


def _get_nc():
    if "nc" not in _STATE:
        os.environ.setdefault("JAX_COMPILATION_CACHE_DIR", "/root/.jax_cache")
        _STATE["nc"] = _build_module()
    return _STATE["nc"]


def _make_in_maps(x, Centroids, Sigmas):
    cwbt_host, cwf_host = _host_precompute(
        np.asarray(Centroids, np.float32), np.asarray(Sigmas, np.float32)
    )
    # rotations pre-scaled by 1/4 (exact in bf16): products land at p/16,
    # matching the x16 coefficient scaling -> single-PSUM accumulation
    xT = np.ascontiguousarray(np.asarray(x, np.float32).T * 0.25).astype(bf16)
    in_maps = []
    for cidx in range(NCORES):
        xTs = np.ascontiguousarray(xT[:, cidx * BLOC:(cidx + 1) * BLOC])
        xrot = np.concatenate(
            [np.roll(xTs, -r, axis=0) for r in ROTVALS], axis=1)   # [D, 15*BLOC]
        hp = np.concatenate(
            [xTs * np.roll(xTs, -j, axis=0) for j in HOST_DIAGS], axis=1)
        in_maps.append({
            "xrot": np.ascontiguousarray(xrot),
            "cwbt": cwbt_host,
            "cwf": cwf_host,
            "hp": np.ascontiguousarray(hp).astype(ml_dtypes.float8_e5m2),
        })
    return in_maps


def _run_device(in_maps, trace=False):
    from concourse import bass_utils

    nc = _get_nc()
    return bass_utils.run_bass_kernel_spmd(
        nc, in_maps, core_ids=list(range(NCORES)), trace=trace
    )


def kernel(x, Centroids, Sigmas):
    in_maps = _make_in_maps(x, Centroids, Sigmas)
    res = _run_device(in_maps)
    outT = np.concatenate([res.results[c]["out"] for c in range(NCORES)], axis=1)
    return np.ascontiguousarray(outT.T).astype(np.float32)


# revision 15
# speedup vs baseline: 1.0544x; 1.0544x over previous
"""Trainium2 Bass kernel for the DEN (Mahalanobis distance) layer.

Computes out[b, e] = (x_b - c_e)^T Sigma_e^{-1} (x_b - c_e) for
x [8192, 128], Centroids [128, 1, 128], Sigmas [128, 128, 128].

Strategy (v7: fp16-highbyte fp8 DoubleRow, arrival-ordered cover)
-----------------------------------------------------------------
Wrapped-diagonal decomposition.  The 64 off-diagonal product packs are
elementwise products of rotated copies of x; a 15-slot difference cover
(rotations 0..7 and 16,24,...,64; diagonal 8 comes from the 16x24 pair)
is DMAed as bf16, pre-scaled by 1/4 on the host so products land at p/16.

DVE writes 56 packs as fp16 at the 2x-mode rate (~0.55 ns/elem) in 20
small batches ordered to match DMA chunk arrival; the idle GpSimd writes
the last 8 (diagonals 57..64).  The PE reads pack HIGH BYTES through a
stride-2 fp8e5 view (fp16 truncation == e5m2) and consumes PAIRS of packs
per fp8 DoubleRow matmul with e4m3 coefficients (x16, CCOMP compensates
the truncation bias).  Everything accumulates into one f32 PSUM group
(2 banks) together with the bf16 linear/x^2 packs; Act evicts bank 0 with
bias=tv while the bank-1 tail finishes, DVE evicts bank 1.

Sharding: data-parallel over batch B across the 8 cores (1024 rows each);
coefficients replicated.
"""

import os
import sys

sys.path.insert(0, "/opt/trn_rl_repo")

import numpy as np
import ml_dtypes

E, B, D = 128, 8192, 128
NCORES = 8
BLOC = B // NCORES          # 1024 batch rows per core
BT = 512                    # matmul free-dim tile (one PSUM bank)
NSLOT = 11                  # rotation slots 0..7, then 16,24,32
ROTVALS = tuple(range(8)) + (16, 24, 32)
NWARM = 5
NPAIR = 32                  # 64 off-diag packs -> 32 DoubleRow pairs
CCOMP = 1.06                # e5m2 truncation compensation (host-tuned)

# DVE product batches (in0 slot range [lo,hi) packed, in1 slot broadcast),
# ordered by DMA chunk arrival.  qSP carries slots 0,2,4,6,14; qAct carries
# 1,8,3,9,5,10,7,11,12,13 (one 256KB chunk each, interleaved with b-slots).
DVE_BATCHES = [
    (1, 2, 0),      # diag 1
    (0, 2, 8),      # 16,15
    (2, 3, 0),      # 2
    (2, 3, 8),      # 14
    (3, 4, 0),      # 3
    (4, 5, 0),      # 4
    (0, 3, 9),      # 24,23,22
    (8, 9, 9),      # 8   (16x24)
    (8, 10, 3),     # 13,21
    (0, 4, 10),     # 32..29
    (8, 11, 4),     # 12,20,28
    (5, 6, 0),      # 5
    (8, 11, 5),     # 11,19,27
    (6, 7, 0),      # 6
    (8, 11, 6),     # 10,18,26
    (7, 8, 0),      # 7
    (8, 11, 7),     # 9,17,25
]
# 32 packs shipped from host as round-nearest fp8e5 (pairs 16..31)
HOST_DIAGS = list(range(33, 65))
N_HOST = len(HOST_DIAGS)

PAIRS_AB = []
for _lo, _hi, _s1 in DVE_BATCHES:
    for _i in range(_lo, _hi):
        PAIRS_AB.append((ROTVALS[_i], ROTVALS[_s1]))
PAIRS_AB += [(0, j) for j in HOST_DIAGS]
assert len(PAIRS_AB) == 2 * NPAIR
_djs = sorted(min((b - a) % 128, (a - b) % 128) for a, b in PAIRS_AB)
assert _djs == list(range(1, 65)), _djs
N_DVE = sum(hi - lo for lo, hi, _ in DVE_BATCHES)       # 64

bf16 = ml_dtypes.bfloat16
f8e4m3 = ml_dtypes.float8_e4m3

_STATE: dict = {}


def _build_module():
    import concourse.bacc as bacc
    import concourse.tile as tile
    import concourse.mybir as mybir
    from contextlib import ExitStack

    nc = bacc.Bacc("TRN2", target_bir_lowering=False, debug=False)

    xr_d = nc.dram_tensor("xrot", [D, NSLOT * BLOC], mybir.dt.bfloat16,
                          kind="ExternalInput")
    # cwbt: [linear(E) | diag(E) | tv as 2 bf16 halves] per partition row
    cwbt_d = nc.dram_tensor("cwbt", [D, 2 * E + 2], mybir.dt.bfloat16,
                            kind="ExternalInput")
    cwf_d = nc.dram_tensor("cwf", [D, NPAIR * 2 * E], mybir.dt.float8e4,
                           kind="ExternalInput")
    hp_d = nc.dram_tensor("hp", [D, N_HOST * BLOC], mybir.dt.float8e5,
                          kind="ExternalInput")
    out_d = nc.dram_tensor("out", [E, BLOC], mybir.dt.bfloat16,
                           kind="ExternalOutput")

    f32 = mybir.dt.float32
    b16 = mybir.dt.bfloat16
    f16 = mybir.dt.float16
    f8e5 = mybir.dt.float8e5
    Ident = mybir.ActivationFunctionType.Identity
    DR = mybir.MatmulPerfMode.DoubleRow

    with tile.TileContext(nc) as tc, ExitStack() as ctx:
        const_pool = ctx.enter_context(tc.tile_pool(name="const", bufs=1))
        psum_pool = ctx.enter_context(tc.tile_pool(name="acc", bufs=2, space="PSUM"))

        # PE warmup on a GpSimd-memset tile: no DMA dependency, trips the
        # clock gate during the DMA prologue.
        WU = const_pool.tile([D, BT], b16, tag="warm")
        nc.gpsimd.memset(WU[:, :], 0)
        PSW = psum_pool.tile([E, BT], f32, tag="psw", name="psw")
        for _ in range(NWARM):
            nc.tensor.matmul(PSW[:, :], WU[:, 0:E], WU[:, :],
                             start=True, stop=True, skip_group_check=True)

        ROTS = const_pool.tile([D, NSLOT * BLOC], b16, tag="rots")
        R3 = ROTS[:, :].rearrange("p (s b) -> p s b", s=NSLOT)
        H = const_pool.tile([D, 32 * BLOC], f16, tag="H")
        H3 = H[:, :].rearrange("p (k b) -> p k b", k=32)
        X2 = const_pool.tile([D, BLOC], b16, tag="x2")
        CWBT = const_pool.tile([D, 2 * E + 2], b16, tag="cwbt")
        TV = CWBT[:, 2 * E:2 * E + 2].bitcast(f32)      # [128, 1] f32 bias
        CWF = const_pool.tile([D, NPAIR * 2 * E], mybir.dt.float8e4, tag="cwf")
        CWF4 = CWF[:, :].rearrange("p (t two e) -> p t two e", t=NPAIR, two=2)
        OT = const_pool.tile([E, BLOC], b16, tag="ot")
        HP = const_pool.tile([D, N_HOST * BLOC], f8e5, tag="hp")
        HP4 = HP[:, :].rearrange("p (t two n) -> p t two n", t=N_HOST // 2, two=2)

        # stride-2 fp8e5 view of H high bytes: [D, pack, col, (lo,hi)] -> hi
        He5 = H[:, :].bitcast(f8e5)
        HV = He5.rearrange("p (k b s) -> p k b s", k=32, s=2)[:, :, :, 1:2]

        def dma_rots(eng, lo, hi):
            eng.dma_start(ROTS[:, lo * BLOC:hi * BLOC],
                          xr_d.ap()[:, lo * BLOC:hi * BLOC])

        # DMA schedule: single-slot chunks, interleaved so DVE batches
        # unlock in arrival order.
        c0 = 8 * 2 * E          # cwf pairs 0..7
        c1 = 16 * 2 * E         # cwf pairs 8..15
        dma_rots(nc.sync, 0, 1)
        dma_rots(nc.scalar, 1, 2)
        dma_rots(nc.sync, 2, 3)
        dma_rots(nc.scalar, 8, 9)
        dma_rots(nc.sync, 4, 5)
        dma_rots(nc.scalar, 3, 4)
        dma_rots(nc.sync, 6, 7)
        dma_rots(nc.scalar, 9, 10)
        nc.sync.dma_start(CWF[:, 0:c0], cwf_d.ap()[:, 0:c0])
        dma_rots(nc.scalar, 5, 6)
        nc.sync.dma_start(CWBT[:, :], cwbt_d.ap())
        dma_rots(nc.scalar, 10, 11)
        nc.sync.dma_start(CWF[:, c0:c1], cwf_d.ap()[:, c0:c1])
        dma_rots(nc.scalar, 7, 8)
        nc.sync.dma_start(CWF[:, c1:], cwf_d.ap()[:, c1:])
        h8 = 8 * BLOC
        nc.scalar.dma_start(HP[:, 0:h8], hp_d.ap()[:, 0:h8])
        nc.sync.dma_start(HP[:, h8:2 * h8], hp_d.ap()[:, h8:2 * h8])
        nc.scalar.dma_start(HP[:, 2 * h8:3 * h8], hp_d.ap()[:, 2 * h8:3 * h8])
        nc.sync.dma_start(HP[:, 3 * h8:], hp_d.ap()[:, 3 * h8:])

        # x^2 on the otherwise-idle GpSimd
        nc.gpsimd.tensor_mul(X2[:, :], ROTS[:, 0:BLOC], ROTS[:, 0:BLOC])

        PS = psum_pool.tile([E, BLOC], f32, tag="ps", name="ps")

        pos = 0         # next pack position not yet consumed by the PE
        hpos = [N_DVE // 2]          # next host pair to emit

        NDP = N_DVE // 2            # DVE-fed pairs

        def emit_host_pair(t, halves, stop):
            for h in halves:
                rhs = HP4[:, t - NDP, :, h * BT:(h + 1) * BT]
                nc.tensor.matmul(PS[:, h * BT:(h + 1) * BT],
                                 CWF4[:, t], rhs,
                                 start=False, stop=stop, perf_mode=DR)

        def emit_pair(t, halves, stop):
            for h in halves:
                rhs = HV[:, 2 * t:2 * t + 2,
                         h * BT:(h + 1) * BT, :].rearrange(
                             "p two b one -> p two (b one)")
                nc.tensor.matmul(PS[:, h * BT:(h + 1) * BT],
                                 CWF4[:, t], rhs,
                                 start=(t == 0),
                                 stop=stop, perf_mode=DR)

        def emit_pairs_until(limit):
            nonlocal pos
            while pos + 2 <= limit:
                t = pos // 2
                emit_pair(t, (0, 1), stop=False)
                pos += 2
                if pos >= 22 and hpos[0] < N_DVE // 2 + 8:
                    emit_host_pair(hpos[0], (0, 1), stop=False)
                    hpos[0] += 1
                if pos == 8:
                    # linear pack (rhs = x/4) mid-chain
                    for bt in range(2):
                        nc.tensor.matmul(PS[:, bt * BT:(bt + 1) * BT],
                                         CWBT[:, 0:E],
                                         ROTS[:, bt * BT:(bt + 1) * BT],
                                         start=False, stop=False)
                if pos == 16:
                    # x^2 pack mid-chain
                    for bt in range(2):
                        nc.tensor.matmul(PS[:, bt * BT:(bt + 1) * BT],
                                         CWBT[:, E:2 * E],
                                         X2[:, bt * BT:(bt + 1) * BT],
                                         start=False, stop=False)

        gpos = 0
        for lo, hi, s1 in DVE_BATCHES:
            w = hi - lo
            nc.vector.tensor_mul(
                H3[:, gpos:gpos + w, :],
                R3[:, lo:hi, :],
                R3[:, s1:s1 + 1, :].broadcast_to((D, w, BLOC)),
            )
            gpos += w
            emit_pairs_until(gpos)
        assert gpos == N_DVE
        # remaining host-fed pairs: all h0s first (bank 0 closes early,
        # eviction overlaps the h1 tail), then all h1s
        for t in range(hpos[0], NPAIR):
            emit_host_pair(t, (0,), stop=(t == NPAIR - 1))

        # bank 0 evicts on Act while bank-1 h1s run, bank 1 on DVE
        nc.scalar.activation(OT[:, 0:BT], PS[:, 0:BT], Ident, bias=TV[:, 0:1])
        nc.sync.dma_start(out_d.ap()[:, 0:BT], OT[:, 0:BT])
        for t in range(hpos[0], NPAIR):
            emit_host_pair(t, (1,), stop=(t == NPAIR - 1))
        nc.vector.tensor_scalar_add(OT[:, BT:BLOC], PS[:, BT:BLOC], TV[:, 0:1])
        nc.scalar.dma_start(out_d.ap()[:, BT:BLOC], OT[:, BT:BLOC])

    nc.compile()
    return nc


def _host_precompute(Centroids: np.ndarray, Sigmas: np.ndarray):
    Sinv = np.linalg.inv(Sigmas.astype(np.float64))
    A = 0.5 * (Sinv + np.swapaxes(Sinv, 1, 2))          # [E, D, D]
    c = Centroids[:, 0, :].astype(np.float64)           # [E, D]
    Ac = np.einsum("edk,ek->ed", A, c)

    idx = np.arange(D)
    cwb = np.zeros((D, 2 * E + 2), np.float32)
    cwb[:, 0:E] = (-2.0 * 4.0 * Ac.T)                   # rhs is x/4
    cwb[:, E:2 * E] = 16.0 * A[:, idx, idx].T           # rhs is (x/4)^2
    cwbt_host = np.ascontiguousarray(cwb).astype(bf16)
    tv = np.einsum("ed,ed->e", Ac, c).astype(np.float32)
    cwbt_host[:, 2 * E:2 * E + 2] = np.ascontiguousarray(
        tv[:, None]).view(bf16).reshape(D, 2)

    cwf = np.zeros((D, 64, E), np.float32)
    for k, (a, b) in enumerate(PAIRS_AB):
        dj = (b - a) % 128
        s = 1.0 if dj == 64 else 2.0
        comp = CCOMP if k < N_DVE else 1.0      # host packs are round-nearest
        cwf[:, k, :] = (s * 16.0 * comp) * A[:, (idx + a) % D,
                                             (idx + b) % D].T
    cwf_host = np.ascontiguousarray(cwf.reshape(D, NPAIR * 2 * E)).astype(f8e4m3)
    return cwbt_host, cwf_host


def _get_nc():
    if "nc" not in _STATE:
        os.environ.setdefault("JAX_COMPILATION_CACHE_DIR", "/root/.jax_cache")
        _STATE["nc"] = _build_module()
    return _STATE["nc"]


def _make_in_maps(x, Centroids, Sigmas):
    cwbt_host, cwf_host = _host_precompute(
        np.asarray(Centroids, np.float32), np.asarray(Sigmas, np.float32)
    )
    # rotations pre-scaled by 1/4 (exact in bf16): products land at p/16,
    # matching the x16 coefficient scaling -> single-PSUM accumulation
    xT = np.ascontiguousarray(np.asarray(x, np.float32).T * 0.25).astype(bf16)
    in_maps = []
    for cidx in range(NCORES):
        xTs = np.ascontiguousarray(xT[:, cidx * BLOC:(cidx + 1) * BLOC])
        xrot = np.concatenate(
            [np.roll(xTs, -r, axis=0) for r in ROTVALS], axis=1)   # [D, 15*BLOC]
        hp = np.concatenate(
            [xTs * np.roll(xTs, -j, axis=0) for j in HOST_DIAGS], axis=1)
        in_maps.append({
            "xrot": np.ascontiguousarray(xrot),
            "cwbt": cwbt_host,
            "cwf": cwf_host,
            "hp": np.ascontiguousarray(hp).astype(ml_dtypes.float8_e5m2),
        })
    return in_maps


def _run_device(in_maps, trace=False):
    from concourse import bass_utils

    nc = _get_nc()
    return bass_utils.run_bass_kernel_spmd(
        nc, in_maps, core_ids=list(range(NCORES)), trace=trace
    )


def kernel(x, Centroids, Sigmas):
    in_maps = _make_in_maps(x, Centroids, Sigmas)
    res = _run_device(in_maps)
    outT = np.concatenate([res.results[c]["out"] for c in range(NCORES)], axis=1)
    return np.ascontiguousarray(outT.T).astype(np.float32)


# revision 16
# speedup vs baseline: 1.1266x; 1.0685x over previous
"""Trainium2 Bass kernel for the DEN (Mahalanobis distance) layer.

Computes out[b, e] = (x_b - c_e)^T Sigma_e^{-1} (x_b - c_e) for
x [8192, 128], Centroids [128, 1, 128], Sigmas [128, 128, 128].

Strategy (v7: fp16-highbyte fp8 DoubleRow, arrival-ordered cover)
-----------------------------------------------------------------
Wrapped-diagonal decomposition.  The 64 off-diagonal product packs are
elementwise products of rotated copies of x; a 15-slot difference cover
(rotations 0..7 and 16,24,...,64; diagonal 8 comes from the 16x24 pair)
is DMAed as bf16, pre-scaled by 1/4 on the host so products land at p/16.

DVE writes 56 packs as fp16 at the 2x-mode rate (~0.55 ns/elem) in 20
small batches ordered to match DMA chunk arrival; the idle GpSimd writes
the last 8 (diagonals 57..64).  The PE reads pack HIGH BYTES through a
stride-2 fp8e5 view (fp16 truncation == e5m2) and consumes PAIRS of packs
per fp8 DoubleRow matmul with e4m3 coefficients (x16, CCOMP compensates
the truncation bias).  Everything accumulates into one f32 PSUM group
(2 banks) together with the bf16 linear/x^2 packs; Act evicts bank 0 with
bias=tv while the bank-1 tail finishes, DVE evicts bank 1.

Sharding: data-parallel over batch B across the 8 cores (1024 rows each);
coefficients replicated.
"""

import os
import sys

sys.path.insert(0, "/opt/trn_rl_repo")

import numpy as np
import ml_dtypes

E, B, D = 128, 8192, 128
NCORES = 8
BLOC = B // NCORES          # 1024 batch rows per core
BT = 512                    # matmul free-dim tile (one PSUM bank)
NSLOT = 11                  # rotation slots 0..7, then 16,24,32
ROTVALS = tuple(range(8)) + (16, 24, 32)
NWARM = 5
NPAIR = 32                  # 64 off-diag packs -> 32 DoubleRow pairs
CCOMP = 1.06                # e5m2 truncation compensation (host-tuned)

# DVE product batches (in0 slot range [lo,hi) packed, in1 slot broadcast),
# ordered by DMA chunk arrival.  qSP carries slots 0,2,4,6,14; qAct carries
# 1,8,3,9,5,10,7,11,12,13 (one 256KB chunk each, interleaved with b-slots).
DVE_BATCHES = [
    (1, 2, 0),      # diag 1
    (0, 2, 8),      # 16,15
    (2, 3, 0),      # 2
    (2, 3, 8),      # 14
    (3, 4, 0),      # 3
    (4, 5, 0),      # 4
    (0, 3, 9),      # 24,23,22
    (8, 9, 9),      # 8   (16x24)
    (8, 10, 3),     # 13,21
    (0, 4, 10),     # 32..29
    (8, 11, 4),     # 12,20,28
    (5, 6, 0),      # 5
    (8, 11, 5),     # 11,19,27
    (6, 7, 0),      # 6
    (8, 11, 6),     # 10,18,26
    (7, 8, 0),      # 7
    (8, 11, 7),     # 9,17,25
]
# 32 packs shipped from host as round-nearest fp8e5 (pairs 16..31)
HOST_DIAGS = list(range(33, 65))
N_HOST = len(HOST_DIAGS)

PAIRS_AB = []
for _lo, _hi, _s1 in DVE_BATCHES:
    for _i in range(_lo, _hi):
        PAIRS_AB.append((ROTVALS[_i], ROTVALS[_s1]))
PAIRS_AB += [(0, j) for j in HOST_DIAGS]
assert len(PAIRS_AB) == 2 * NPAIR
_djs = sorted(min((b - a) % 128, (a - b) % 128) for a, b in PAIRS_AB)
assert _djs == list(range(1, 65)), _djs
N_DVE = sum(hi - lo for lo, hi, _ in DVE_BATCHES)       # 64

bf16 = ml_dtypes.bfloat16
f8e4m3 = ml_dtypes.float8_e4m3

_STATE: dict = {}


def _build_module():
    import concourse.bacc as bacc
    import concourse.tile as tile
    import concourse.mybir as mybir
    from contextlib import ExitStack

    nc = bacc.Bacc("TRN2", target_bir_lowering=False, debug=False)

    xr_d = nc.dram_tensor("xrot", [D, NSLOT * BLOC], mybir.dt.bfloat16,
                          kind="ExternalInput")
    # cwbt: [linear(E) | diag(E) | tv as 2 bf16 halves] per partition row
    cwbt_d = nc.dram_tensor("cwbt", [D, 2 * E + 2], mybir.dt.bfloat16,
                            kind="ExternalInput")
    cwf_d = nc.dram_tensor("cwf", [D, NPAIR * 2 * E], mybir.dt.float8e4,
                           kind="ExternalInput")
    hp_d = nc.dram_tensor("hp", [D, N_HOST * BLOC], mybir.dt.float8e5,
                          kind="ExternalInput")
    out_d = nc.dram_tensor("out", [E, BLOC], mybir.dt.bfloat16,
                           kind="ExternalOutput")

    f32 = mybir.dt.float32
    b16 = mybir.dt.bfloat16
    f16 = mybir.dt.float16
    f8e5 = mybir.dt.float8e5
    Ident = mybir.ActivationFunctionType.Identity
    DR = mybir.MatmulPerfMode.DoubleRow

    with tile.TileContext(nc) as tc, ExitStack() as ctx:
        const_pool = ctx.enter_context(tc.tile_pool(name="const", bufs=1))
        psum_pool = ctx.enter_context(tc.tile_pool(name="acc", bufs=2, space="PSUM"))

        # PE warmup on a GpSimd-memset tile: no DMA dependency, trips the
        # clock gate during the DMA prologue.
        WU = const_pool.tile([D, BT], b16, tag="warm")
        nc.gpsimd.memset(WU[:, :], 0)
        PSW = psum_pool.tile([E, BT], f32, tag="psw", name="psw")
        for _ in range(NWARM):
            nc.tensor.matmul(PSW[:, :], WU[:, 0:E], WU[:, :],
                             start=True, stop=True, skip_group_check=True)

        ROTS = const_pool.tile([D, NSLOT * BLOC], b16, tag="rots")
        R3 = ROTS[:, :].rearrange("p (s b) -> p s b", s=NSLOT)
        H = const_pool.tile([D, 32 * BLOC], f16, tag="H")
        H3 = H[:, :].rearrange("p (k b) -> p k b", k=32)
        X2 = const_pool.tile([D, BLOC], b16, tag="x2")
        CWBT = const_pool.tile([D, 2 * E + 2], b16, tag="cwbt")
        TV = CWBT[:, 2 * E:2 * E + 2].bitcast(f32)      # [128, 1] f32 bias
        CWF = const_pool.tile([D, NPAIR * 2 * E], mybir.dt.float8e4, tag="cwf")
        CWF4 = CWF[:, :].rearrange("p (t two e) -> p t two e", t=NPAIR, two=2)
        OT = const_pool.tile([E, BLOC], b16, tag="ot")
        HP = const_pool.tile([D, N_HOST * BLOC], f8e5, tag="hp")
        HP4 = HP[:, :].rearrange("p (t two n) -> p t two n", t=N_HOST // 2, two=2)

        # stride-2 fp8e5 view of H high bytes: [D, pack, col, (lo,hi)] -> hi
        He5 = H[:, :].bitcast(f8e5)
        HV = He5.rearrange("p (k b s) -> p k b s", k=32, s=2)[:, :, :, 1:2]

        def dma_rots(eng, lo, hi):
            eng.dma_start(ROTS[:, lo * BLOC:hi * BLOC],
                          xr_d.ap()[:, lo * BLOC:hi * BLOC])

        # DMA schedule: single-slot chunks, interleaved so DVE batches
        # unlock in arrival order.
        c0 = 8 * 2 * E          # cwf pairs 0..7
        c1 = 16 * 2 * E         # cwf pairs 8..15
        dma_rots(nc.sync, 0, 1)
        dma_rots(nc.scalar, 1, 2)
        dma_rots(nc.sync, 2, 3)
        dma_rots(nc.scalar, 8, 9)
        dma_rots(nc.sync, 4, 5)
        dma_rots(nc.scalar, 3, 4)
        dma_rots(nc.sync, 6, 7)
        dma_rots(nc.scalar, 9, 10)
        nc.sync.dma_start(CWF[:, 0:c0], cwf_d.ap()[:, 0:c0])
        dma_rots(nc.scalar, 5, 6)
        nc.sync.dma_start(CWBT[:, :], cwbt_d.ap())
        dma_rots(nc.scalar, 10, 11)
        nc.sync.dma_start(CWF[:, c0:c1], cwf_d.ap()[:, c0:c1])
        dma_rots(nc.scalar, 7, 8)
        nc.sync.dma_start(CWF[:, c1:], cwf_d.ap()[:, c1:])
        h8 = 8 * BLOC
        nc.scalar.dma_start(HP[:, 0:h8], hp_d.ap()[:, 0:h8])
        nc.sync.dma_start(HP[:, h8:2 * h8], hp_d.ap()[:, h8:2 * h8])
        nc.scalar.dma_start(HP[:, 2 * h8:3 * h8], hp_d.ap()[:, 2 * h8:3 * h8])
        nc.sync.dma_start(HP[:, 3 * h8:], hp_d.ap()[:, 3 * h8:])

        # x^2 on the otherwise-idle GpSimd
        nc.gpsimd.tensor_mul(X2[:, :], ROTS[:, 0:BLOC], ROTS[:, 0:BLOC])

        PS = psum_pool.tile([E, BLOC], f32, tag="ps", name="ps")

        pos = 0         # next pack position not yet consumed by the PE
        hpos = [N_DVE // 2]          # next host pair to emit

        NDP = N_DVE // 2            # DVE-fed pairs

        def emit_host_pair(t, halves, stop):
            for h in halves:
                rhs = HP4[:, t - NDP, :, h * BT:(h + 1) * BT]
                nc.tensor.matmul(PS[:, h * BT:(h + 1) * BT],
                                 CWF4[:, t], rhs,
                                 start=False, stop=stop, perf_mode=DR)

        def emit_pair(t, halves, stop):
            for h in halves:
                rhs = HV[:, 2 * t:2 * t + 2,
                         h * BT:(h + 1) * BT, :].rearrange(
                             "p two b one -> p two (b one)")
                nc.tensor.matmul(PS[:, h * BT:(h + 1) * BT],
                                 CWF4[:, t], rhs,
                                 start=(t == 0),
                                 stop=stop, perf_mode=DR)

        def emit_pairs_until(limit):
            nonlocal pos
            while pos + 2 <= limit:
                t = pos // 2
                emit_pair(t, (0, 1), stop=False)
                pos += 2
                if pos >= 22 and hpos[0] < N_DVE // 2 + 8:
                    emit_host_pair(hpos[0], (0, 1), stop=False)
                    hpos[0] += 1
                if pos == 8:
                    # linear pack (rhs = x/4) mid-chain
                    for bt in range(2):
                        nc.tensor.matmul(PS[:, bt * BT:(bt + 1) * BT],
                                         CWBT[:, 0:E],
                                         ROTS[:, bt * BT:(bt + 1) * BT],
                                         start=False, stop=False)
                if pos == 16:
                    # x^2 pack mid-chain
                    for bt in range(2):
                        nc.tensor.matmul(PS[:, bt * BT:(bt + 1) * BT],
                                         CWBT[:, E:2 * E],
                                         X2[:, bt * BT:(bt + 1) * BT],
                                         start=False, stop=False)

        gpos = 0
        for lo, hi, s1 in DVE_BATCHES:
            w = hi - lo
            nc.vector.tensor_mul(
                H3[:, gpos:gpos + w, :],
                R3[:, lo:hi, :],
                R3[:, s1:s1 + 1, :].broadcast_to((D, w, BLOC)),
            )
            gpos += w
            emit_pairs_until(gpos)
        assert gpos == N_DVE
        # remaining host-fed pairs; the last one closes both banks
        for t in range(hpos[0], NPAIR):
            emit_host_pair(t, (0,), stop=(t == NPAIR - 1))
            emit_host_pair(t, (1,), stop=(t == NPAIR - 1))

        # bank 0 evicts on Act, bank 1 on DVE
        nc.scalar.activation(OT[:, 0:BT], PS[:, 0:BT], Ident, bias=TV[:, 0:1])
        nc.sync.dma_start(out_d.ap()[:, 0:BT], OT[:, 0:BT])
        nc.vector.tensor_scalar_add(OT[:, BT:BLOC], PS[:, BT:BLOC], TV[:, 0:1])
        nc.scalar.dma_start(out_d.ap()[:, BT:BLOC], OT[:, BT:BLOC])

    nc.compile()
    return nc


def _host_precompute(Centroids: np.ndarray, Sigmas: np.ndarray):
    Sinv = np.linalg.inv(Sigmas.astype(np.float64))
    A = 0.5 * (Sinv + np.swapaxes(Sinv, 1, 2))          # [E, D, D]
    c = Centroids[:, 0, :].astype(np.float64)           # [E, D]
    Ac = np.einsum("edk,ek->ed", A, c)

    idx = np.arange(D)
    cwb = np.zeros((D, 2 * E + 2), np.float32)
    cwb[:, 0:E] = (-2.0 * 4.0 * Ac.T)                   # rhs is x/4
    cwb[:, E:2 * E] = 16.0 * A[:, idx, idx].T           # rhs is (x/4)^2
    cwbt_host = np.ascontiguousarray(cwb).astype(bf16)
    tv = np.einsum("ed,ed->e", Ac, c).astype(np.float32)
    cwbt_host[:, 2 * E:2 * E + 2] = np.ascontiguousarray(
        tv[:, None]).view(bf16).reshape(D, 2)

    cwf = np.zeros((D, 64, E), np.float32)
    for k, (a, b) in enumerate(PAIRS_AB):
        dj = (b - a) % 128
        s = 1.0 if dj == 64 else 2.0
        comp = CCOMP if k < N_DVE else 1.0      # host packs are round-nearest
        cwf[:, k, :] = (s * 16.0 * comp) * A[:, (idx + a) % D,
                                             (idx + b) % D].T
    cwf_host = np.ascontiguousarray(cwf.reshape(D, NPAIR * 2 * E)).astype(f8e4m3)
    return cwbt_host, cwf_host


def _get_nc():
    if "nc" not in _STATE:
        os.environ.setdefault("JAX_COMPILATION_CACHE_DIR", "/root/.jax_cache")
        _STATE["nc"] = _build_module()
    return _STATE["nc"]


def _make_in_maps(x, Centroids, Sigmas):
    cwbt_host, cwf_host = _host_precompute(
        np.asarray(Centroids, np.float32), np.asarray(Sigmas, np.float32)
    )
    # rotations pre-scaled by 1/4 (exact in bf16): products land at p/16,
    # matching the x16 coefficient scaling -> single-PSUM accumulation
    xT = np.ascontiguousarray(np.asarray(x, np.float32).T * 0.25).astype(bf16)
    in_maps = []
    for cidx in range(NCORES):
        xTs = np.ascontiguousarray(xT[:, cidx * BLOC:(cidx + 1) * BLOC])
        xrot = np.concatenate(
            [np.roll(xTs, -r, axis=0) for r in ROTVALS], axis=1)   # [D, 15*BLOC]
        hp = np.concatenate(
            [xTs * np.roll(xTs, -j, axis=0) for j in HOST_DIAGS], axis=1)
        in_maps.append({
            "xrot": np.ascontiguousarray(xrot),
            "cwbt": cwbt_host,
            "cwf": cwf_host,
            "hp": np.ascontiguousarray(hp).astype(ml_dtypes.float8_e5m2),
        })
    return in_maps


def _run_device(in_maps, trace=False):
    from concourse import bass_utils

    nc = _get_nc()
    return bass_utils.run_bass_kernel_spmd(
        nc, in_maps, core_ids=list(range(NCORES)), trace=trace
    )


def kernel(x, Centroids, Sigmas):
    in_maps = _make_in_maps(x, Centroids, Sigmas)
    res = _run_device(in_maps)
    outT = np.concatenate([res.results[c]["out"] for c in range(NCORES)], axis=1)
    return np.ascontiguousarray(outT.T).astype(np.float32)
